# revision 6
# baseline (speedup 1.0000x reference)
"""Trainium2 Bass kernel for AttnBlock (GroupNorm + 1x1-conv QKV self-attention + proj + residual).

Input x: (2, 256, 64, 64) f32.  8 NeuronCores, SPMD: core = b*4 + iq handles
batch b and query pixels [iq*1024, (iq+1)*1024) of the 4096-pixel image.
Pixel-axis orderings are permutation-invariant, so the host rolls each core's
pixel axis to put its own queries at columns 0:1024 - one SPMD program.

v2 structure (from the v1 trace: 105us = 39us preamble + 49us PE-bound
attention + 23us tail, with PE at half clock outside [42,93]us):

  1. DMA count minimized (8 x-chunks of [128,1024], 1 packed weight DMA,
     1 bf16 wp DMA, 1 packed selector DMA, 1 store per query half) - the
     v1 x stream was DMA-issue-rate bound on the sync engine.
  2. PE HAM clock-gate warmup: dummy f32 matmuls from t=0 (memset tile,
     then each arriving x chunk) keep the PE activity monitor hot so the
     fold chain + projections run at 2.4GHz instead of 1.2GHz.
  3. ACT table prewarm: dummy Sqrt then dummy Exp at boot load both table
     sets off the critical path; all later activations (rstd sqrt, exps)
     hit loaded tables.  The only other set load (Reciprocal, for the
     half-1 softmax denominator) is issued after the last exp and hides
     behind the dT1 accumulation wait.
  4. Per-channel stats via bn_stats on [128,1024] chunks as they arrive,
     combined into 32 group stats with selector matmuls; GroupNorm folded
     into the QKV weights (wX_eff = wX^T * gamma*rstd, bias folded); the
     attention scale 1/sqrt(C) is pre-folded into wq on the host.
  5. x cast f32->fp8 on GpSimd (idle engine; v1 spent 11.5us of ScalarE).
  6. All projections + attention matmuls fp8-e4m3 DoubleRow.  QK writes
     score PAIRS into [128,2,512] 2-bank PSUM tiles; ONE exp activation
     per pair (32 total instead of 64 - the 352-cycle ACT fixed overhead
     was 19us of the v1 runtime).  bk cancels under softmax and is dropped.
  7. The softmax denominator dT accumulates per key-pair during BOTH
     halves (ones-stationary DR matmul).  Half-0's entire normalize tail
     (1/d, broadcast via GpSimd partition_broadcast, wp projection,
     residual add, store) is emitted right after half-0 so it overlaps
     half-1's attention.  Half-0 1/d on DVE (hidden); half-1 1/d on ACT
     Reciprocal (0.7us vs 4.3us DVE serial on one partition).

Validated end-to-end rel err ~4e-4 vs the fp32 reference (fp8 rounding is
attenuated because the residual x dominates the output).
"""

import sys

sys.path.insert(0, "/opt/trn_rl_repo")

import numpy as np
import ml_dtypes

import concourse.bass as bass
import concourse.tile as tile
from concourse import bacc, mybir
from concourse.bass_utils import run_bass_kernel_spmd

F32 = mybir.dt.float32
BF16 = mybir.dt.bfloat16
FP8 = mybir.dt.float8e4
DR = mybir.MatmulPerfMode.DoubleRow
AF = mybir.ActivationFunctionType
ALU = mybir.AluOpType

C = 256  # channels
N = 4096  # pixels (64*64)
NQ = 1024  # query pixels per core
NG = 32  # groups
EPS = 1e-6


def build_bass():
    nc = bacc.Bacc("TRN2", target_bir_lowering=False, debug=False)

    x_d = nc.declare_dram_parameter("x", [C, N], F32, isOutput=False)
    # wpack cols: 0:256 wqT*scale, 256:512 wkT, 512:768 wvT,
    # 768:774 smalls (bq*scale, bk, bv, bp, gamma, beta)
    wpack_d = nc.declare_dram_parameter("wpack", [128, 2, 774], F32, isOutput=False)
    wpT_d = nc.declare_dram_parameter("wpT", [128, 2, C], BF16, isOutput=False)
    # selc: [:, 0:16] = sel1 (channel->group avg), [0:64, 16:272] = sel2
    selc_d = nc.declare_dram_parameter("selc", [128, 272], F32, isOutput=False)
    out_d = nc.declare_dram_parameter("out", [128, 2, NQ], F32, isOutput=True)

    with tile.TileContext(nc) as tc:
        with (
            tc.tile_pool(name="consts", bufs=1) as consts,
            tc.tile_pool(name="big", bufs=1) as big,
            tc.tile_pool(name="stats", bufs=1) as stats,
            tc.tile_pool(name="work", bufs=2) as work,
            tc.tile_pool(name="psS", bufs=2, space="PSUM") as psS,
            tc.tile_pool(name="psO", bufs=1, space="PSUM") as psO,
            tc.tile_pool(name="psD", bufs=1, space="PSUM") as psD,
            tc.tile_pool(name="psT", bufs=1, space="PSUM") as psT,
        ):
            # ---------------- boot: warmups ----------------
            # HAM clock-gate warmup: PE boots throttled to half clock and
            # only ramps after ~4us of sustained activity; dummy matmuls
            # keep it hot until the real stream starts.
            warm = consts.tile([128, 512], F32)
            nc.vector.memset(warm[:, :], 1.0)
            eps32 = consts.tile([64, 1], F32)
            nc.vector.memset(eps32[:, :], EPS)
            for w in range(3):
                psw = psT.tile([128, 512], F32, tag="ps", name=f"warm{w}")
                nc.tensor.matmul(
                    psw[:, :], warm[:, 0:128], warm[:, :], start=True, stop=True
                )
            # ACT table prewarm: load sqrt set then exp set at boot so the
            # rstd sqrt (fold path) and the exp stream hit loaded tables.
            wact = stats.tile([1, 2], F32)
            nc.scalar.activation(
                out=wact[:, 0:1], in_=eps32[0:1, :], func=AF.Sqrt, bias=0.0, scale=1.0
            )

            # ---------------- x load + per-channel stats ----------------
            sel1 = consts.tile([128, 16], F32)
            sel2 = consts.tile([64, C], F32)
            selc = consts.tile([128, 272], F32)

            grp = stats.tile([64, 8], F32)
            nc.vector.memset(grp[:, :], 1.0)

            x_f = big.tile([128, 2, N], F32)
            x_b = big.tile([128, 2, N], FP8)
            bn6 = stats.tile([128, 2, 8, 6], F32)
            stat2 = stats.tile([128, 2, 2], F32)
            msq = stats.tile([128, 2, 1], F32)
            # cc-outer order so the query-pixel chunks (cc=0) land first
            for cc in range(4):
                cs = slice(cc * 1024, (cc + 1) * 1024)
                for h in range(2):
                    r = slice(h * 128, (h + 1) * 128)
                    nc.sync.dma_start(out=x_f[:, h, cs], in_=x_d[r, cs])
                    for s2 in range(2):
                        c512 = slice(cc * 1024 + s2 * 512, cc * 1024 + (s2 + 1) * 512)
                        nc.vector.bn_stats(
                            out=bn6[:, h, cc * 2 + s2, :], in_=x_f[:, h, c512]
                        )
                    nc.gpsimd.tensor_copy(out=x_b[:, h, cs], in_=x_f[:, h, cs])
                    # HAM keep-alive, paced by chunk arrival (f32, FD=128)
                    psw = psT.tile([128, 512], F32, tag="ps", name=f"warmx{cc}_{h}")
                    nc.tensor.matmul(
                        psw[:, 0:128],
                        x_f[:, h, cc * 1024 : cc * 1024 + 128],
                        x_f[:, h, cc * 1024 : cc * 1024 + 128],
                        start=True,
                        stop=True,
                    )
            # selector + weight loads queue behind x (small; arrive ~3us later)
            nc.sync.dma_start(out=selc[:, :], in_=selc_d[:, :])
            wpack = consts.tile([128, 2, 774], F32)
            wpT_b = consts.tile([128, 2, C], BF16)
            nc.sync.dma_start(out=wpack[:, :, :], in_=wpack_d[:, :, :])
            nc.sync.dma_start(out=wpT_b[:, :, :], in_=wpT_d[:, :, :])
            # fp32 matmuls fuse the weight load and carry one sync wait, so
            # selector operands must come from the DVE sem domain: bounce.
            nc.vector.tensor_copy(out=sel1[:, :], in_=selc[:, 0:16])
            nc.vector.tensor_copy(out=sel2[:, :], in_=selc[0:64, 16:272])

            for h in range(2):
                nc.vector.bn_aggr(out=stat2[:, h, :], in_=bn6[:, h, :, :])
                nc.vector.tensor_scalar_mul(
                    msq[:, h, :], stat2[:, h, 0:1], stat2[:, h, 0:1]
                )
                nc.vector.tensor_scalar_add(
                    stat2[:, h, 1:2], stat2[:, h, 1:2], msq[:, h, :]
                )
                # group stats: groups 0-15 at partitions 0-15, 16-31 at 32-47
                psg = psT.tile([128, 512], F32, tag="ps", name=f"psg{h}")
                nc.tensor.matmul(
                    psg[0:16, 0:2], sel1[:, :], stat2[:, h, :], start=True, stop=True
                )
                nc.vector.tensor_copy(out=grp[h * 32 : h * 32 + 16, 0:2], in_=psg[0:16, 0:2])

            # grp cols: 0=mean 1=E[x2]->rstd 2=mean^2 3=var 4=sqrt(var+eps)
            nc.vector.tensor_scalar_mul(grp[:, 2:3], grp[:, 0:1], grp[:, 0:1])
            nc.vector.tensor_scalar_sub(grp[:, 3:4], grp[:, 1:2], grp[:, 2:3])
            nc.scalar.activation(
                out=grp[:, 4:5], in_=grp[:, 3:4], func=AF.Sqrt, bias=eps32[:, :], scale=1.0
            )
            nc.vector.reciprocal(out=grp[:, 1:2], in_=grp[:, 4:5])
            # prewarm the exp table now - ACT is idle, fold chain is on DVE/PE
            nc.scalar.activation(
                out=wact[:, 1:2], in_=wact[:, 0:1], func=AF.Exp, bias=0.0, scale=1.0
            )

            # expand to per-channel: mr[:, h, 0]=mean_bc, mr[:, h, 1]=rstd_bc
            smalls = wpack[:, :, 768:774]
            mr = stats.tile([128, 2, 2], F32)
            sc = stats.tile([128, 2, 1], F32)
            tsh = stats.tile([128, 2, 1], F32)
            for h in range(2):
                pse = psT.tile([128, 512], F32, tag="ps", name=f"pse{h}")
                nc.tensor.matmul(
                    pse[:, 0:2],
                    sel2[:, h * 128 : (h + 1) * 128],
                    grp[:, 0:2],
                    start=True,
                    stop=True,
                )
                nc.vector.tensor_copy(out=mr[:, h, :], in_=pse[:, 0:2])
                # s = gamma * rstd ; t = beta - s*mean
                nc.vector.tensor_scalar_mul(sc[:, h, :], smalls[:, h, 4:5], mr[:, h, 1:2])
                nc.vector.tensor_scalar_mul(tsh[:, h, :], sc[:, h, :], mr[:, h, 0:1])
                nc.vector.tensor_sub(tsh[:, h, :], smalls[:, h, 5:6], tsh[:, h, :])

            # ---------------- fold norm into weights (fp8) ----------------
            wqT_e = consts.tile([128, 2, C], FP8)
            wkT_e = consts.tile([128, 2, C], FP8)
            wvT_e = consts.tile([128, 2, C], FP8)
            stv = stats.tile([128, 2, 1], F32)
            for h in range(2):
                nc.vector.tensor_scalar_mul(wqT_e[:, h, :], wpack[:, h, 0:256], sc[:, h, :])
                nc.vector.tensor_scalar_mul(wkT_e[:, h, :], wpack[:, h, 256:512], sc[:, h, :])
                nc.vector.tensor_scalar_mul(wvT_e[:, h, :], wpack[:, h, 512:768], sc[:, h, :])
                nc.vector.tensor_scalar_mul(stv[:, h, :], tsh[:, h, :], sc[:, h, :])

            # effective biases: bXe[o] = bX[o] + sum_c wXT_e[c,o] * (s*t)[c]
            # (k needs no bias: scoresT[j,i] += bk.q_i is constant over j and
            # softmax is shift-invariant, so bk cancels exactly)
            stv8 = stats.tile([128, 2, 1], FP8)
            for h in range(2):
                nc.vector.tensor_copy(out=stv8[:, h, :], in_=stv[:, h, :])
            bqe = stats.tile([128, 2, 1], F32)
            bve = stats.tile([128, 2, 1], F32)
            bvb = stats.tile([128, 2, 1], BF16)
            for (we, bs, bo) in ((wqT_e, 0, bqe), (wvT_e, 2, bve)):
                for o in range(2):
                    psb = psT.tile([128, 512], F32, tag="ps", name=f"psb{bs}_{o}")
                    for h in range(2):
                        nc.tensor.matmul(
                            psb[:, 0:1],
                            we[:, h, o * 128 : (o + 1) * 128],
                            stv8[:, h, :],
                            start=(h == 0),
                            stop=(h == 1),
                        )
                    nc.vector.tensor_scalar_add(bo[:, o, :], psb[:, 0:1], smalls[:, o, bs : bs + 1])
            for o in range(2):
                nc.vector.tensor_copy(out=bvb[:, o, :], in_=bve[:, o, :])
            # bpe[o] = bp[o] + sum_c wpT[c, o] * bve[c]
            bpe = stats.tile([128, 2, 1], F32)
            for o in range(2):
                psb = psT.tile([128, 512], F32, tag="ps", name=f"psbp{o}")
                for h in range(2):
                    nc.tensor.matmul(
                        psb[:, 0:1],
                        wpT_b[:, h, o * 128 : (o + 1) * 128],
                        bvb[:, h, :],
                        start=(h == 0),
                        stop=(h == 1),
                    )
                nc.vector.tensor_scalar_add(bpe[:, o, :], psb[:, 0:1], smalls[:, o, 3:4])

            # fp8 ones for the DR softmax-denominator matmuls (Ko stride 16B)
            ones8 = consts.tile([128, 2, 16], FP8)
            nc.vector.memset(ones8[:, :, :], 1.0)

            # ------- fused fp8-DR projections + query-half-0 attention -------
            q_b = big.tile([128, 2, NQ], FP8)
            for ch in range(2):
                cs = slice(ch * 512, (ch + 1) * 512)
                for o in range(2):
                    psq = psS.tile([128, 2, 512], F32, tag="pss", name=f"psq{ch}_{o}")
                    nc.tensor.matmul(
                        psq[:, 0, :], wqT_e[:, :, o * 128 : (o + 1) * 128],
                        x_b[:, :, cs], start=True, stop=True, perf_mode=DR,
                    )
                    nc.vector.tensor_scalar_add(q_b[:, o, cs], psq[:, 0, :], bqe[:, o, :])

            k_b = big.tile([128, 2, N], FP8)
            vT_b = big.tile([128, 32, 272], FP8)
            dT0 = psD.tile([1, 512], F32, tag="dt", name="dT0")
            pso0 = psO.tile([128, 2, 512], F32, tag="pso", name="pso0")
            for ch in range(8):
                cs = slice(ch * 512, (ch + 1) * 512)
                for o in range(2):
                    psk = psS.tile([128, 2, 512], F32, tag="pss", name=f"psk{ch}_{o}")
                    nc.tensor.matmul(
                        psk[:, 0, :], wkT_e[:, :, o * 128 : (o + 1) * 128],
                        x_b[:, :, cs], start=True, stop=True, perf_mode=DR,
                    )
                    nc.vector.tensor_copy(out=k_b[:, o, cs], in_=psk[:, 0, :])
                for jj in range(4):
                    j = ch * 4 + jj
                    psv = psS.tile([128, 2, 512], F32, tag="pss", name=f"psv{j}")
                    nc.tensor.matmul(
                        psv[:, 0, 0:C], x_b[:, :, j * 128 : (j + 1) * 128],
                        wvT_e[:, :, :], start=True, stop=True, perf_mode=DR,
                    )
                    nc.vector.tensor_copy(out=vT_b[:, j, 0:C], in_=psv[:, 0, 0:C])
                for jp in (ch * 2, ch * 2 + 1):
                    eT2 = work.tile([128, 2, 512], FP8, tag="expT", bufs=4, name=f"eT0_{jp}")
                    pss = psS.tile([128, 2, 512], F32, tag="pss", name=f"pss0_{jp}")
                    for par in range(2):
                        j = jp * 2 + par
                        nc.tensor.matmul(
                            pss[:, par, :], k_b[:, :, j * 128 : (j + 1) * 128],
                            q_b[:, :, 0:512], start=True, stop=True, perf_mode=DR,
                        )
                    nc.scalar.activation(
                        out=eT2[:, :, :], in_=pss[:, :, :], func=AF.Exp,
                        bias=0.0, scale=1.0,
                    )
                    for o in range(2):
                        nc.tensor.matmul(
                            pso0[:, o, :],
                            vT_b[:, 2 * jp : 2 * jp + 2, o * 128 : (o + 1) * 128],
                            eT2[:, :, :],
                            start=(jp == 0), stop=(jp == 15), perf_mode=DR,
                        )
                    nc.tensor.matmul(
                        dT0[:, :], ones8[:, :, 0:1], eT2[:, :, :],
                        start=(jp == 0), stop=(jp == 15), perf_mode=DR,
                    )

            # -------- half-0 normalize tail (overlaps half-1 attention) ------
            # DVE-queue order matters: o2s0/d0sb first so the pso0/dT0 banks
            # free for half-1 before the 4.3us serial reciprocal runs.
            o2s0 = work.tile([128, 2, 512], BF16, tag="o2s", bufs=2, name="o2s0")
            for o in range(2):
                nc.vector.tensor_copy(out=o2s0[:, o, :], in_=pso0[:, o, :])
            d0sb = work.tile([1, 512], F32, tag="dsb", bufs=2, name="d0sb")
            nc.vector.tensor_copy(out=d0sb[:, :], in_=dT0[:, :])
            # residual base (needs only bpe + the query-quarter of x)
            xres = big.tile([128, 2, NQ], F32)
            for h in range(2):
                nc.vector.tensor_scalar_add(xres[:, h, :], x_f[:, h, 0:NQ], bpe[:, h, :])
            # 1/d0 on DVE (hidden under half-1); broadcast on GpSimd.
            recd0 = work.tile([1, 512], F32, tag="recd", bufs=2, name="recd0")
            nc.vector.reciprocal(out=recd0[:, :], in_=d0sb[:, :])
            bca0 = work.tile([128, 512], F32, tag="bca", bufs=2, name="bca0")
            nc.gpsimd.partition_broadcast(out_ap=bca0[:, :], in_ap=recd0[:, :], channels=128)
            fin0 = work.tile([128, 2, 512], F32, tag="fin", bufs=2, name="fin0")
            for o in range(2):
                psp = psT.tile([128, 512], F32, tag="ps", name=f"psp0_{o}")
                for ch2 in range(2):
                    nc.tensor.matmul(
                        psp[:, :],
                        wpT_b[:, ch2, o * 128 : (o + 1) * 128],
                        o2s0[:, ch2, :],
                        start=(ch2 == 0),
                        stop=(ch2 == 1),
                    )
                fmul = work.tile([128, 512], F32, tag="fmul", bufs=2, name=f"fmul0_{o}")
                nc.vector.tensor_mul(fmul[:, :], psp[:, :], bca0[:, :])
                nc.vector.tensor_add(fin0[:, o, :], fmul[:, :], xres[:, o, 0:512])
            nc.sync.dma_start(out=out_d[:, :, 0:512], in_=fin0[:, :, :])

            # ---------------- query-half-1 attention ----------------
            pso1 = psO.tile([128, 2, 512], F32, tag="pso", name="pso1")
            dT1 = psD.tile([1, 512], F32, tag="dt", name="dT1")
            for jp in range(16):
                eT2 = work.tile([128, 2, 512], FP8, tag="expT", bufs=4, name=f"eT1_{jp}")
                pss = psS.tile([128, 2, 512], F32, tag="pss", name=f"pss1_{jp}")
                for par in range(2):
                    j = jp * 2 + par
                    nc.tensor.matmul(
                        pss[:, par, :], k_b[:, :, j * 128 : (j + 1) * 128],
                        q_b[:, :, 512:1024], start=True, stop=True, perf_mode=DR,
                    )
                nc.scalar.activation(
                    out=eT2[:, :, :], in_=pss[:, :, :], func=AF.Exp,
                    bias=0.0, scale=1.0,
                )
                for o in range(2):
                    nc.tensor.matmul(
                        pso1[:, o, :],
                        vT_b[:, 2 * jp : 2 * jp + 2, o * 128 : (o + 1) * 128],
                        eT2[:, :, :],
                        start=(jp == 0), stop=(jp == 15), perf_mode=DR,
                    )
                nc.tensor.matmul(
                    dT1[:, :], ones8[:, :, 0:1], eT2[:, :, :],
                    start=(jp == 0), stop=(jp == 15), perf_mode=DR,
                )
            # half-1 tail
            d1sb = work.tile([1, 512], F32, tag="dsb", bufs=2, name="d1sb")
            nc.vector.tensor_copy(out=d1sb[:, :], in_=dT1[:, :])
            recd1 = work.tile([1, 512], F32, tag="recd", bufs=2, name="recd1")
            nc.vector.reciprocal(out=recd1[:, :], in_=d1sb[:, :])
            bca1 = work.tile([128, 512], F32, tag="bca", bufs=2, name="bca1")
            nc.gpsimd.partition_broadcast(out_ap=bca1[:, :], in_ap=recd1[:, :], channels=128)
            # ACT is idle once the last exp drains: final out2 copy goes there
            o2s1 = work.tile([128, 2, 512], BF16, tag="o2s", bufs=2, name="o2s1")
            for o in range(2):
                nc.scalar.activation(
                    out=o2s1[:, o, :], in_=pso1[:, o, :], func=AF.Copy,
                    bias=0.0, scale=1.0,
                )
            fin1 = work.tile([128, 2, 512], F32, tag="fin", bufs=2, name="fin1")
            for o in range(2):
                psp = psT.tile([128, 512], F32, tag="ps", name=f"psp1_{o}")
                for ch2 in range(2):
                    nc.tensor.matmul(
                        psp[:, :],
                        wpT_b[:, ch2, o * 128 : (o + 1) * 128],
                        o2s1[:, ch2, :],
                        start=(ch2 == 0),
                        stop=(ch2 == 1),
                    )
                fmul = work.tile([128, 512], F32, tag="fmul", bufs=2, name=f"fmul1_{o}")
                nc.vector.tensor_mul(fmul[:, :], psp[:, :], bca1[:, :])
                nc.vector.tensor_add(fin1[:, o, :], fmul[:, :], xres[:, o, 512:1024])
            nc.sync.dma_start(out=out_d[:, :, 512:1024], in_=fin1[:, :, :])
    nc.compile()
    return nc


_NC_CACHE = None


def _get_nc():
    global _NC_CACHE
    if _NC_CACHE is None:
        _NC_CACHE = build_bass()
    return _NC_CACHE


def make_in_maps(inputs):
    x = np.asarray(inputs["x"], dtype=np.float32)
    scale = C ** (-0.5)
    wqT = np.ascontiguousarray((np.asarray(inputs["wq"]) * scale).T.astype(np.float32))
    wkT = np.ascontiguousarray(np.asarray(inputs["wk"]).T.astype(np.float32))
    wvT = np.ascontiguousarray(np.asarray(inputs["wv"]).T.astype(np.float32))
    smalls = np.stack(
        [
            np.asarray(inputs["bq"]) * scale,
            np.asarray(inputs["bk"]),
            np.asarray(inputs["bv"]),
            np.asarray(inputs["bp"]),
            np.asarray(inputs["norm_gamma"]),
            np.asarray(inputs["norm_beta"]),
        ],
        axis=1,
    ).astype(np.float32)
    wcat = np.concatenate([wqT, wkT, wvT, smalls], axis=1)  # [256, 774]
    wpack = np.ascontiguousarray(wcat.reshape(2, 128, 774).transpose(1, 0, 2))
    wpT = np.asarray(inputs["wp"]).T.astype(ml_dtypes.bfloat16)
    wpT = np.ascontiguousarray(wpT.reshape(2, 128, C).transpose(1, 0, 2))

    cidx = np.arange(C)
    selc = np.zeros((128, 272), np.float32)
    selc[np.arange(128), np.arange(128) // 8] = 1.0 / 8.0
    # group g lives at partition g (g<16) or 32+g-16 (g>=16)
    grow = np.where(cidx // 8 < 16, cidx // 8, 32 + cidx // 8 - 16)
    sel2 = np.zeros((64, C), np.float32)
    sel2[grow, cidx] = 1.0
    selc[0:64, 16:272] = sel2

    common = dict(wpack=wpack, wpT=wpT, selc=selc)
    in_maps = []
    for core in range(8):
        b, iq = core // 4, core % 4
        xb = x[b].reshape(C, N)
        xr = np.ascontiguousarray(np.roll(xb, -iq * NQ, axis=1))
        in_maps.append(dict(common, x=xr))
    return in_maps


def assemble_output(results, like):
    out = np.empty((2, C, N), np.float32)
    for core in range(8):
        b, iq = core // 4, core % 4
        o = results[core]["out"]  # [128, 2, NQ]
        out[b][:, iq * NQ : (iq + 1) * NQ] = o.transpose(1, 0, 2).reshape(C, NQ)
    return out.reshape(like.shape).astype(np.float32)


def kernel(**inputs):
    nc = _get_nc()
    in_maps = make_in_maps(inputs)
    res = run_bass_kernel_spmd(nc, in_maps, core_ids=list(range(8)))
    return assemble_output(res.results, np.asarray(inputs["x"]))


def kernel_traced(inputs, **kwargs):
    """test-only helper: returns (output, BassKernelResults with exec_time_ns)."""
    nc = _get_nc()
    in_maps = make_in_maps(inputs)
    res = run_bass_kernel_spmd(nc, in_maps, core_ids=list(range(8)), trace=True, **kwargs)
    return assemble_output(res.results, np.asarray(inputs["x"])), res


# revision 14
# speedup vs baseline: 1.0891x; 1.0891x over previous
"""Trainium2 Bass kernel for AttnBlock (GroupNorm + 1x1-conv QKV self-attention + proj + residual).

Input x: (2, 256, 64, 64) f32.  8 NeuronCores, SPMD: core = b*4 + iq handles
batch b and query pixels [iq*1024, (iq+1)*1024) of the 4096-pixel image.
Pixel-axis orderings are permutation-invariant, so the host rolls each core's
pixel axis to put its own queries at columns 0:1024 - one SPMD program.

v2 structure (from the v1 trace: 105us = 39us preamble + 49us PE-bound
attention + 23us tail, with PE at half clock outside [42,93]us):

  1. DMA count minimized (8 x-chunks of [128,1024], 1 packed weight DMA,
     1 bf16 wp DMA, 1 packed selector DMA, 1 store per query half) - the
     v1 x stream was DMA-issue-rate bound on the sync engine.
  2. PE HAM clock-gate warmup: dummy f32 matmuls from t=0 (memset tile,
     then each arriving x chunk) keep the PE activity monitor hot so the
     fold chain + projections run at 2.4GHz instead of 1.2GHz.
  3. ACT table prewarm: dummy Sqrt then dummy Exp at boot load both table
     sets off the critical path; all later activations (rstd sqrt, exps)
     hit loaded tables.  The only other set load (Reciprocal, for the
     half-1 softmax denominator) is issued after the last exp and hides
     behind the dT1 accumulation wait.
  4. Per-channel stats via bn_stats on [128,1024] chunks as they arrive,
     combined into 32 group stats with selector matmuls; GroupNorm folded
     into the QKV weights (wX_eff = wX^T * gamma*rstd, bias folded); the
     attention scale 1/sqrt(C) is pre-folded into wq on the host.
  5. x cast f32->fp8 on GpSimd (idle engine; v1 spent 11.5us of ScalarE).
  6. All projections + attention matmuls fp8-e4m3 DoubleRow.  QK writes
     score PAIRS into [128,2,512] 2-bank PSUM tiles; ONE exp activation
     per pair (32 total instead of 64 - the 352-cycle ACT fixed overhead
     was 19us of the v1 runtime).  bk cancels under softmax and is dropped.
  7. The softmax denominator dT accumulates per key-pair during BOTH
     halves (ones-stationary DR matmul).  Half-0's entire normalize tail
     (1/d, broadcast via GpSimd partition_broadcast, wp projection,
     residual add, store) is emitted right after half-0 so it overlaps
     half-1's attention.  Half-0 1/d on DVE (hidden); half-1 1/d on ACT
     Reciprocal (0.7us vs 4.3us DVE serial on one partition).

Validated end-to-end rel err ~4e-4 vs the fp32 reference (fp8 rounding is
attenuated because the residual x dominates the output).
"""

import sys

sys.path.insert(0, "/opt/trn_rl_repo")

import numpy as np
import ml_dtypes

import concourse.bass as bass
import concourse.tile as tile
from concourse import bacc, mybir
from concourse.bass_utils import run_bass_kernel_spmd

F32 = mybir.dt.float32
BF16 = mybir.dt.bfloat16
FP8 = mybir.dt.float8e4
DR = mybir.MatmulPerfMode.DoubleRow
AF = mybir.ActivationFunctionType
ALU = mybir.AluOpType

C = 256  # channels
N = 4096  # pixels (64*64)
NQ = 1024  # query pixels per core
NG = 32  # groups
EPS = 1e-6


def build_bass():
    nc = bacc.Bacc("TRN2", target_bir_lowering=False, debug=False)

    x_d = nc.declare_dram_parameter("x", [C, N], F32, isOutput=False)
    # wpack cols: 0:256 wqT*scale, 256:512 wkT, 512:768 wvT,
    # 768:774 smalls (bq*scale, bk, bv, bp, gamma, beta)
    wpack_d = nc.declare_dram_parameter("wpack", [128, 2, 774], F32, isOutput=False)
    wpT_d = nc.declare_dram_parameter("wpT", [128, 2, C], BF16, isOutput=False)
    # selc: [:, 0:16] = sel1 (channel->group avg), [0:64, 16:272] = sel2
    selc_d = nc.declare_dram_parameter("selc", [128, 272], F32, isOutput=False)
    out_d = nc.declare_dram_parameter("out", [128, 2, NQ], F32, isOutput=True)

    with tile.TileContext(nc) as tc:
        with (
            tc.tile_pool(name="consts", bufs=1) as consts,
            tc.tile_pool(name="big", bufs=1) as big,
            tc.tile_pool(name="stats", bufs=1) as stats,
            tc.tile_pool(name="work", bufs=2) as work,
            tc.tile_pool(name="psS", bufs=2, space="PSUM") as psS,
            tc.tile_pool(name="psO", bufs=1, space="PSUM") as psO,
            tc.tile_pool(name="psD", bufs=1, space="PSUM") as psD,
            tc.tile_pool(name="psT", bufs=1, space="PSUM") as psT,
        ):
            # ---------------- boot: warmups ----------------
            # HAM clock-gate warmup: PE boots throttled to half clock and
            # only ramps after ~4us of sustained activity; dummy matmuls
            # keep it hot until the real stream starts.
            warm = consts.tile([128, 512], F32)
            nc.vector.memset(warm[:, :], 1.0)
            eps32 = consts.tile([64, 1], F32)
            nc.vector.memset(eps32[:, :], EPS)
            for w in range(3):
                psw = psT.tile([128, 512], F32, tag="ps", name=f"warm{w}")
                nc.tensor.matmul(
                    psw[:, :], warm[:, 0:128], warm[:, :], start=True, stop=True
                )
            # ACT table prewarm: the whole kernel runs from the
            # natural_log_exp set (ln/exp/copy/identity; see the post-compile
            # retarget in build_bass) - one boot-time load covers everything.
            wact = stats.tile([1, 2], F32)
            nc.scalar.activation(
                out=wact[:, 0:1], in_=eps32[0:1, :], func=AF.Ln, bias=0.0, scale=1.0
            )

            # ---------------- x load + per-channel stats ----------------
            sel1 = consts.tile([128, 16], F32)
            sel2 = consts.tile([64, C], F32)
            selc = consts.tile([128, 272], F32)

            grp = stats.tile([64, 8], F32)
            nc.vector.memset(grp[:, :], 1.0)

            x_f = big.tile([128, 2, N], F32)
            x_b = big.tile([128, 2, N], FP8)
            bn6 = stats.tile([128, 2, 8, 6], F32)
            stat2 = stats.tile([128, 2, 2], F32)
            msq = stats.tile([128, 2, 1], F32)
            # cc-outer order so the query-pixel chunks (cc=0) land first
            for cc in range(4):
                cs = slice(cc * 1024, (cc + 1) * 1024)
                for h in range(2):
                    r = slice(h * 128, (h + 1) * 128)
                    nc.sync.dma_start(out=x_f[:, h, cs], in_=x_d[r, cs])
                    for s2 in range(2):
                        c512 = slice(cc * 1024 + s2 * 512, cc * 1024 + (s2 + 1) * 512)
                        nc.vector.bn_stats(
                            out=bn6[:, h, cc * 2 + s2, :], in_=x_f[:, h, c512]
                        )
                    # f32->fp8 cast rides a gpsimd-issued SWDGE DMA (the DMA
                    # engines convert in flight; gpsimd compute casts are 4x
                    # slower and ACT needs its cycles for the exp stream)
                    nc.gpsimd.dma_start(out=x_b[:, h, cs], in_=x_f[:, h, cs])
                    # HAM keep-alive, paced by chunk arrival (f32, FD=128)
                    psw = psT.tile([128, 512], F32, tag="ps", name=f"warmx{cc}_{h}")
                    nc.tensor.matmul(
                        psw[:, 0:128],
                        x_f[:, h, cc * 1024 : cc * 1024 + 128],
                        x_f[:, h, cc * 1024 : cc * 1024 + 128],
                        start=True,
                        stop=True,
                    )
            # selector + weight loads queue behind x (small; arrive ~3us later)
            nc.sync.dma_start(out=selc[:, :], in_=selc_d[:, :])
            wpack = consts.tile([128, 2, 774], F32)
            wpT_b = consts.tile([128, 2, C], BF16)
            nc.sync.dma_start(out=wpack[:, :, :], in_=wpack_d[:, :, :])
            nc.sync.dma_start(out=wpT_b[:, :, :], in_=wpT_d[:, :, :])
            # fp32 matmuls fuse the weight load and carry one sync wait, so
            # selector operands must come from the DVE sem domain: bounce.
            nc.vector.tensor_copy(out=sel1[:, :], in_=selc[:, 0:16])
            nc.vector.tensor_copy(out=sel2[:, :], in_=selc[0:64, 16:272])

            for h in range(2):
                nc.vector.bn_aggr(out=stat2[:, h, :], in_=bn6[:, h, :, :])
                nc.vector.tensor_scalar_mul(
                    msq[:, h, :], stat2[:, h, 0:1], stat2[:, h, 0:1]
                )
                nc.vector.tensor_scalar_add(
                    stat2[:, h, 1:2], stat2[:, h, 1:2], msq[:, h, :]
                )
                # group stats: groups 0-15 at partitions 0-15, 16-31 at 32-47
                psg = psT.tile([128, 512], F32, tag="ps", name=f"psg{h}")
                nc.tensor.matmul(
                    psg[0:16, 0:2], sel1[:, :], stat2[:, h, :], start=True, stop=True
                )
                nc.vector.tensor_copy(out=grp[h * 32 : h * 32 + 16, 0:2], in_=psg[0:16, 0:2])

            # grp cols: 0=mean 1=E[x2]->rstd 2=mean^2 3=var 4=ln(var+eps)
            # rstd = exp(-0.5*ln(var+eps)) - ln/exp stay in one ACT table set
            nc.vector.tensor_scalar_mul(grp[:, 2:3], grp[:, 0:1], grp[:, 0:1])
            nc.vector.tensor_scalar_sub(grp[:, 3:4], grp[:, 1:2], grp[:, 2:3])
            nc.scalar.activation(
                out=grp[:, 4:5], in_=grp[:, 3:4], func=AF.Ln, bias=eps32[:, :], scale=1.0
            )
            nc.scalar.activation(
                out=grp[:, 1:2], in_=grp[:, 4:5], func=AF.Exp, bias=0.0, scale=-0.5
            )

            # expand to per-channel: mr[:, h, 0]=mean_bc, mr[:, h, 1]=rstd_bc
            smalls = wpack[:, :, 768:774]
            mr = stats.tile([128, 2, 2], F32)
            sc = stats.tile([128, 2, 1], F32)
            tsh = stats.tile([128, 2, 1], F32)
            for h in range(2):
                pse = psT.tile([128, 512], F32, tag="ps", name=f"pse{h}")
                nc.tensor.matmul(
                    pse[:, 0:2],
                    sel2[:, h * 128 : (h + 1) * 128],
                    grp[:, 0:2],
                    start=True,
                    stop=True,
                )
                nc.vector.tensor_copy(out=mr[:, h, :], in_=pse[:, 0:2])
                # s = gamma * rstd ; t = beta - s*mean
                nc.vector.tensor_scalar_mul(sc[:, h, :], smalls[:, h, 4:5], mr[:, h, 1:2])
                nc.vector.tensor_scalar_mul(tsh[:, h, :], sc[:, h, :], mr[:, h, 0:1])
                nc.vector.tensor_sub(tsh[:, h, :], smalls[:, h, 5:6], tsh[:, h, :])

            # ---------------- fold norm into weights (fp8) ----------------
            wqT_e = consts.tile([128, 2, C], FP8)
            wkT_e = consts.tile([128, 2, C], FP8)
            wvT_e = consts.tile([128, 2, C], FP8)
            stv = stats.tile([128, 2, 1], F32)
            for h in range(2):
                nc.vector.tensor_scalar_mul(wqT_e[:, h, :], wpack[:, h, 0:256], sc[:, h, :])
                nc.vector.tensor_scalar_mul(wkT_e[:, h, :], wpack[:, h, 256:512], sc[:, h, :])
                nc.vector.tensor_scalar_mul(wvT_e[:, h, :], wpack[:, h, 512:768], sc[:, h, :])
                nc.vector.tensor_scalar_mul(stv[:, h, :], tsh[:, h, :], sc[:, h, :])

            # effective biases: bXe[o] = bX[o] + sum_c wXT_e[c,o] * (s*t)[c]
            # (k needs no bias: scoresT[j,i] += bk.q_i is constant over j and
            # softmax is shift-invariant, so bk cancels exactly)
            stv8 = stats.tile([128, 2, 1], FP8)
            for h in range(2):
                nc.vector.tensor_copy(out=stv8[:, h, :], in_=stv[:, h, :])
            bqe = stats.tile([128, 2, 1], F32)
            bve = stats.tile([128, 2, 1], F32)
            bvb = stats.tile([128, 2, 1], BF16)
            for (we, bs, bo) in ((wqT_e, 0, bqe), (wvT_e, 2, bve)):
                for o in range(2):
                    psb = psT.tile([128, 512], F32, tag="ps", name=f"psb{bs}_{o}")
                    for h in range(2):
                        nc.tensor.matmul(
                            psb[:, 0:1],
                            we[:, h, o * 128 : (o + 1) * 128],
                            stv8[:, h, :],
                            start=(h == 0),
                            stop=(h == 1),
                        )
                    nc.vector.tensor_scalar_add(bo[:, o, :], psb[:, 0:1], smalls[:, o, bs : bs + 1])
            for o in range(2):
                nc.vector.tensor_copy(out=bvb[:, o, :], in_=bve[:, o, :])
            # bpe[o] = bp[o] + sum_c wpT[c, o] * bve[c]
            bpe = stats.tile([128, 2, 1], F32)
            for o in range(2):
                psb = psT.tile([128, 512], F32, tag="ps", name=f"psbp{o}")
                for h in range(2):
                    nc.tensor.matmul(
                        psb[:, 0:1],
                        wpT_b[:, h, o * 128 : (o + 1) * 128],
                        bvb[:, h, :],
                        start=(h == 0),
                        stop=(h == 1),
                    )
                nc.vector.tensor_scalar_add(bpe[:, o, :], psb[:, 0:1], smalls[:, o, 3:4])

            # fp8 ones for the DR softmax-denominator matmuls (Ko stride 16B)
            ones8 = consts.tile([128, 2, 16], FP8)
            nc.vector.memset(ones8[:, :, :], 1.0)

            # ------- fused fp8-DR projections + query-half-0 attention -------
            q_b = big.tile([128, 2, NQ], FP8)
            for ch in range(2):
                cs = slice(ch * 512, (ch + 1) * 512)
                for o in range(2):
                    psq = psS.tile([128, 2, 512], F32, tag="pss", name=f"psq{ch}_{o}")
                    nc.tensor.matmul(
                        psq[:, 0, :], wqT_e[:, :, o * 128 : (o + 1) * 128],
                        x_b[:, :, cs], start=True, stop=True, perf_mode=DR,
                    )
                    nc.scalar.activation(
                        out=q_b[:, o, cs], in_=psq[:, 0, :], func=AF.Identity,
                        bias=bqe[:, o, :], scale=1.0,
                    )

            k_b = big.tile([128, 2, N], FP8)
            vT_b = big.tile([128, 32, 272], FP8)
            dT0 = psD.tile([1, 512], F32, tag="dt", name="dT0")
            pso0 = psO.tile([128, 2, 512], F32, tag="pso", name="pso0")
            for ch in range(8):
                cs = slice(ch * 512, (ch + 1) * 512)
                for o in range(2):
                    psk = psS.tile([128, 2, 512], F32, tag="pss", name=f"psk{ch}_{o}")
                    nc.tensor.matmul(
                        psk[:, 0, :], wkT_e[:, :, o * 128 : (o + 1) * 128],
                        x_b[:, :, cs], start=True, stop=True, perf_mode=DR,
                    )
                    if ch < 2:
                        # ACT is idle until the exp stream ramps
                        nc.scalar.activation(
                            out=k_b[:, o, cs], in_=psk[:, 0, :], func=AF.Copy,
                            bias=0.0, scale=1.0,
                        )
                    else:
                        nc.vector.tensor_copy(out=k_b[:, o, cs], in_=psk[:, 0, :])
                for jj in range(4):
                    j = ch * 4 + jj
                    psv = psS.tile([128, 2, 512], F32, tag="pss", name=f"psv{j}")
                    nc.tensor.matmul(
                        psv[:, 0, 0:C], x_b[:, :, j * 128 : (j + 1) * 128],
                        wvT_e[:, :, :], start=True, stop=True, perf_mode=DR,
                    )
                    nc.vector.tensor_copy(out=vT_b[:, j, 0:C], in_=psv[:, 0, 0:C])
                for jp in (ch * 2, ch * 2 + 1):
                    eT2 = work.tile([128, 2, 512], FP8, tag="expT", bufs=4, name=f"eT0_{jp}")
                    pss = psS.tile([128, 2, 512], F32, tag="pss", name=f"pss0_{jp}")
                    for par in range(2):
                        j = jp * 2 + par
                        nc.tensor.matmul(
                            pss[:, par, :], k_b[:, :, j * 128 : (j + 1) * 128],
                            q_b[:, :, 0:512], start=True, stop=True, perf_mode=DR,
                        )
                    nc.scalar.activation(
                        out=eT2[:, :, :], in_=pss[:, :, :], func=AF.Exp,
                        bias=0.0, scale=1.0,
                    )
                    for o in range(2):
                        nc.tensor.matmul(
                            pso0[:, o, :],
                            vT_b[:, 2 * jp : 2 * jp + 2, o * 128 : (o + 1) * 128],
                            eT2[:, :, :],
                            start=(jp == 0), stop=(jp == 15), perf_mode=DR,
                        )
                    nc.tensor.matmul(
                        dT0[:, :], ones8[:, :, 0:1], eT2[:, :, :],
                        start=(jp == 0), stop=(jp == 15), perf_mode=DR,
                    )

            # -------- half-0 normalize tail (overlaps half-1 attention) ------
            # 1/d0 = exp(-ln(d0)) on ACT (2 ops in the exp-stream gaps; a DVE
            # reciprocal on [1,512] costs 4.3us serial on one partition).
            lnd0 = work.tile([1, 512], F32, tag="lnd", bufs=2, name="lnd0")
            nc.scalar.activation(
                out=lnd0[:, :], in_=dT0[:, :], func=AF.Ln, bias=0.0, scale=1.0
            )
            recd0 = work.tile([1, 512], F32, tag="recd", bufs=2, name="recd0")
            nc.scalar.activation(
                out=recd0[:, :], in_=lnd0[:, :], func=AF.Exp, bias=0.0, scale=-1.0
            )
            o2s0 = work.tile([128, 2, 512], BF16, tag="o2s", bufs=2, name="o2s0")
            for o in range(2):
                nc.vector.tensor_copy(out=o2s0[:, o, :], in_=pso0[:, o, :])
            # residual base (needs only bpe + the query-quarter of x)
            xres = big.tile([128, 2, NQ], F32)
            for h in range(2):
                nc.vector.tensor_scalar_add(xres[:, h, :], x_f[:, h, 0:NQ], bpe[:, h, :])
            bca0 = work.tile([128, 512], F32, tag="bca", bufs=2, name="bca0")
            nc.gpsimd.partition_broadcast(out_ap=bca0[:, :], in_ap=recd0[:, :], channels=128)
            fin0 = work.tile([128, 2, 512], F32, tag="fin", bufs=2, name="fin0")
            for o in range(2):
                psp = psT.tile([128, 512], F32, tag="ps", name=f"psp0_{o}")
                for ch2 in range(2):
                    nc.tensor.matmul(
                        psp[:, :],
                        wpT_b[:, ch2, o * 128 : (o + 1) * 128],
                        o2s0[:, ch2, :],
                        start=(ch2 == 0),
                        stop=(ch2 == 1),
                    )
                fmul = work.tile([128, 512], F32, tag="fmul", bufs=2, name=f"fmul0_{o}")
                nc.vector.tensor_mul(fmul[:, :], psp[:, :], bca0[:, :])
                nc.vector.tensor_add(fin0[:, o, :], fmul[:, :], xres[:, o, 0:512])
            nc.sync.dma_start(out=out_d[:, :, 0:512], in_=fin0[:, :, :])

            # ---------------- query-half-1 attention ----------------
            pso1 = psO.tile([128, 2, 512], F32, tag="pso", name="pso1")
            dT1 = psD.tile([1, 512], F32, tag="dt", name="dT1")
            for jp in range(16):
                eT2 = work.tile([128, 2, 512], FP8, tag="expT", bufs=4, name=f"eT1_{jp}")
                pss = psS.tile([128, 2, 512], F32, tag="pss", name=f"pss1_{jp}")
                for par in range(2):
                    j = jp * 2 + par
                    nc.tensor.matmul(
                        pss[:, par, :], k_b[:, :, j * 128 : (j + 1) * 128],
                        q_b[:, :, 512:1024], start=True, stop=True, perf_mode=DR,
                    )
                nc.scalar.activation(
                    out=eT2[:, :, :], in_=pss[:, :, :], func=AF.Exp,
                    bias=0.0, scale=1.0,
                )
                for o in range(2):
                    nc.tensor.matmul(
                        pso1[:, o, :],
                        vT_b[:, 2 * jp : 2 * jp + 2, o * 128 : (o + 1) * 128],
                        eT2[:, :, :],
                        start=(jp == 0), stop=(jp == 15), perf_mode=DR,
                    )
                nc.tensor.matmul(
                    dT1[:, :], ones8[:, :, 0:1], eT2[:, :, :],
                    start=(jp == 0), stop=(jp == 15), perf_mode=DR,
                )
            # half-1 tail: 1/d1 = exp(-ln(d1)) on ACT
            lnd1 = work.tile([1, 512], F32, tag="lnd", bufs=2, name="lnd1")
            nc.scalar.activation(
                out=lnd1[:, :], in_=dT1[:, :], func=AF.Ln, bias=0.0, scale=1.0
            )
            recd1 = work.tile([1, 512], F32, tag="recd", bufs=2, name="recd1")
            nc.scalar.activation(
                out=recd1[:, :], in_=lnd1[:, :], func=AF.Exp, bias=0.0, scale=-1.0
            )
            bca1 = work.tile([128, 512], F32, tag="bca", bufs=2, name="bca1")
            nc.gpsimd.partition_broadcast(out_ap=bca1[:, :], in_ap=recd1[:, :], channels=128)
            # ACT is idle once the last exp drains: final out2 copy goes there
            o2s1 = work.tile([128, 2, 512], BF16, tag="o2s", bufs=2, name="o2s1")
            for o in range(2):
                nc.scalar.activation(
                    out=o2s1[:, o, :], in_=pso1[:, o, :], func=AF.Copy,
                    bias=0.0, scale=1.0,
                )
            fin1 = work.tile([128, 2, 512], F32, tag="fin", bufs=2, name="fin1")
            for o in range(2):
                psp = psT.tile([128, 512], F32, tag="ps", name=f"psp1_{o}")
                for ch2 in range(2):
                    nc.tensor.matmul(
                        psp[:, :],
                        wpT_b[:, ch2, o * 128 : (o + 1) * 128],
                        o2s1[:, ch2, :],
                        start=(ch2 == 0),
                        stop=(ch2 == 1),
                    )
                fmul = work.tile([128, 512], F32, tag="fmul", bufs=2, name=f"fmul1_{o}")
                nc.vector.tensor_mul(fmul[:, :], psp[:, :], bca1[:, :])
                nc.vector.tensor_add(fin1[:, o, :], fmul[:, :], xres[:, o, 512:1024])
            nc.sync.dma_start(out=out_d[:, :, 512:1024], in_=fin1[:, :, :])
    nc.compile()
    _consolidate_act_table_loads(nc)
    return nc


# index of natural_log_exp_and_others in act_info.json's act_func_sets -
# the one set containing every activation this kernel uses (ln, exp,
# copy, identity).
_NLE_SET = 6


def _consolidate_act_table_loads(nc):
    """The compiler's table picker greedily selects the first set serving
    each activation (natural_log for Ln, exp_and_others for Exp), inserting
    a ~1.3us reload at every ln<->exp transition.  Every activation here is
    served by the combined natural_log_exp set, so retarget the first load
    and drop the redundant reloads.  Loads are inserted after semaphore
    generation and carry no sync state (asserted below), so removal is a
    pure ACT-queue shortening."""
    first = True
    for b in nc.main_func.blocks:
        keep = []
        for i in b.instructions:
            if isinstance(i, mybir.InstLoadActFuncSet):
                si = i.sync_info
                assert si is None or (len(si.on_wait) == 0 and len(si.on_update) == 0), (
                    "act table load unexpectedly carries sync info"
                )
                if first:
                    i.act_func_set_id = _NLE_SET
                    first = False
                    keep.append(i)
                continue
            keep.append(i)
        if len(keep) != len(b.instructions):
            b.instructions[:] = keep


_NC_CACHE = None


def _get_nc():
    global _NC_CACHE
    if _NC_CACHE is None:
        _NC_CACHE = build_bass()
    return _NC_CACHE


def make_in_maps(inputs):
    x = np.asarray(inputs["x"], dtype=np.float32)
    scale = C ** (-0.5)
    wqT = np.ascontiguousarray((np.asarray(inputs["wq"]) * scale).T.astype(np.float32))
    wkT = np.ascontiguousarray(np.asarray(inputs["wk"]).T.astype(np.float32))
    wvT = np.ascontiguousarray(np.asarray(inputs["wv"]).T.astype(np.float32))
    smalls = np.stack(
        [
            np.asarray(inputs["bq"]) * scale,
            np.asarray(inputs["bk"]),
            np.asarray(inputs["bv"]),
            np.asarray(inputs["bp"]),
            np.asarray(inputs["norm_gamma"]),
            np.asarray(inputs["norm_beta"]),
        ],
        axis=1,
    ).astype(np.float32)
    wcat = np.concatenate([wqT, wkT, wvT, smalls], axis=1)  # [256, 774]
    wpack = np.ascontiguousarray(wcat.reshape(2, 128, 774).transpose(1, 0, 2))
    wpT = np.asarray(inputs["wp"]).T.astype(ml_dtypes.bfloat16)
    wpT = np.ascontiguousarray(wpT.reshape(2, 128, C).transpose(1, 0, 2))

    cidx = np.arange(C)
    selc = np.zeros((128, 272), np.float32)
    selc[np.arange(128), np.arange(128) // 8] = 1.0 / 8.0
    # group g lives at partition g (g<16) or 32+g-16 (g>=16)
    grow = np.where(cidx // 8 < 16, cidx // 8, 32 + cidx // 8 - 16)
    sel2 = np.zeros((64, C), np.float32)
    sel2[grow, cidx] = 1.0
    selc[0:64, 16:272] = sel2

    common = dict(wpack=wpack, wpT=wpT, selc=selc)
    in_maps = []
    for core in range(8):
        b, iq = core // 4, core % 4
        xb = x[b].reshape(C, N)
        xr = np.ascontiguousarray(np.roll(xb, -iq * NQ, axis=1))
        in_maps.append(dict(common, x=xr))
    return in_maps


def assemble_output(results, like):
    out = np.empty((2, C, N), np.float32)
    for core in range(8):
        b, iq = core // 4, core % 4
        o = results[core]["out"]  # [128, 2, NQ]
        out[b][:, iq * NQ : (iq + 1) * NQ] = o.transpose(1, 0, 2).reshape(C, NQ)
    return out.reshape(like.shape).astype(np.float32)


def kernel(**inputs):
    nc = _get_nc()
    in_maps = make_in_maps(inputs)
    res = run_bass_kernel_spmd(nc, in_maps, core_ids=list(range(8)))
    return assemble_output(res.results, np.asarray(inputs["x"]))


def kernel_traced(inputs, **kwargs):
    """test-only helper: returns (output, BassKernelResults with exec_time_ns)."""
    nc = _get_nc()
    in_maps = make_in_maps(inputs)
    res = run_bass_kernel_spmd(nc, in_maps, core_ids=list(range(8)), trace=True, **kwargs)
    return assemble_output(res.results, np.asarray(inputs["x"])), res
